# revision 13
# baseline (speedup 1.0000x reference)
"""Trainium2 Bass kernel for nn_CvxpyProjectionLayer.

Solves 2048 independent small QPs (projection with slack penalty) by an
OSQP-style ADMM, data-parallel over 8 NeuronCores (256 elements/core,
2 partition-tiles of 128 batch elements in partitions).

Algorithm notes:
 - The reference's 150 ADMM iterations (rho=1, alpha=1.6) are fully
   converged (ref@150 vs ref@600 < 1e-6), so we converge to the same fixed
   point faster: rho=2.0, alpha=1.7 hits the fp32 noise floor (~6e-5
   absmax vs the jax reference, full batch) in 46 iterations. The tiny
   sigma=1e-6 proximal terms are dropped (validated: no accuracy change).
 - KKT matrix K = rho*M^T M + diag(p)+sigma in closed form:
       K = [[rho*(A^T A) + (rho+1+sig)I , -rho*A_s^T],
            [-rho*A_s                   , (2rho+2+sig)I]]
 - Per-iteration batched matvecs (a different matrix per batch element)
   are spread over three engine rails:
     * A^T u, A xt: tensor_tensor multiply with broadcast APs, rows split
       between DVE and GPSIMD, + segmented tensor_reduce (DVE) or row-wise
       activation-accumulate reduce (ScalarE).
     * Kinv rhs: fused multiply-accumulate chain on DVE
       (scalar_tensor_tensor with per-partition scalars), one op per
       contraction index, reading contiguous columns of a column-major
       Kinv layout.
   Elementwise state updates go to GPSIMD/ScalarE/DVE.
"""

import sys
from concurrent.futures import ThreadPoolExecutor

import numpy as np

sys.path.insert(0, "/opt/trn_rl_repo")

import concourse.bacc as bacc  # noqa: E402
import concourse.mybir as mybir  # noqa: E402
from concourse.bass_utils import run_bass_kernel_spmd  # noqa: E402
from concourse.tile import TileContext  # noqa: E402

NCORES = 8
PER_CORE = 256
T = 2
P = 128
n = 80
m = 85
ns = 4
N = n + ns  # 84

RHO = 2.0
ALPHA = 1.7
SIGMA = 1e-6
PEN = 1.0
ITERS = 46

# --- engine assignment knobs (tuned against TimelineSim) ---
DVI_AT = 40        # A^T u mult: i-rows [0, DVI_AT) on DVE, rest GPSIMD
DVJ_AX = 42        # A xt mult: rows [0, DVJ_AX) on DVE, rest GPSIMD
ATU_RED = "dve"  # A^T u reduce: "dve" | "act"
AXT_RED = "dve"    # A xt reduce: "dve" | "act"
KINV_MODE = "mr"  # "stt": DVE MAC chain | "mr": mult(split)+reduce
KINV_GP_ROWS = 42   # when "mr": how many of the 84 i-rows multiply on GPSIMD

U_ON = "gp"

f32 = mybir.dt.float32
OP = mybir.AluOpType
AX = mybir.AxisListType
AF = mybir.ActivationFunctionType


def _emit_matvec_mult(nc, tmp_view, a_view, vec_bc, rows_dve, rows_total):
    """tmp[p, r, c] = a[p, r, c] * vec[p, c], rows split DVE/GPSIMD."""
    if rows_dve > 0:
        nc.vector.tensor_tensor(tmp_view[:, 0:rows_dve, :],
                                a_view[:, 0:rows_dve, :],
                                vec_bc[:, 0:rows_dve, :], op=OP.mult)
    if rows_dve < rows_total:
        nc.gpsimd.tensor_tensor(tmp_view[:, rows_dve:rows_total, :],
                                a_view[:, rows_dve:rows_total, :],
                                vec_bc[:, rows_dve:rows_total, :], op=OP.mult)


def _emit_reduce(nc, out, tmp_view, rows, where, dump):
    """out[p, r] = sum_c tmp[p, r, c], on DVE (one op) or ACT (row loop)."""
    if where == "dve":
        nc.vector.tensor_reduce(out, tmp_view, axis=AX.X, op=OP.add)
    else:
        for r in range(rows):
            nc.scalar.activation(dump[:, 0:tmp_view.shape[2]],
                                 tmp_view[:, r, :], AF.Copy,
                                 accum_out=out[:, r:r + 1])


def _emit_iteration(nc, st, cst, scr):
    v = nc.vector
    gp = nc.gpsimd
    sc = nc.scalar

    x, zA, zs, zb, YA, Ys, Yb = (
        st["x"], st["zA"], st["zs"], st["zb"], st["YA"], st["Ys"], st["Yb"])
    xn, zAn, zsn, zbn, YAn, Ysn, Ybn = (
        st["xn"], st["zAn"], st["zsn"], st["zbn"], st["YAn"], st["Ysn"], st["Ybn"])

    uA, us, ub = scr["uA"], scr["us"], scr["ub"]
    tmp = scr["tmp"]
    mA = scr["mA"]
    rhs = scr["rhs"]
    xt = scr["xt"]
    xts = scr["xts"]
    ztA = scr["ztA"]
    ztAs = scr["ztAs"]
    zrA, zrs, zrb = scr["zrA"], scr["zrs"], scr["zrb"]
    wA, ws, wb = scr["wA"], scr["ws"], scr["wb"]
    t80, t4 = scr["t80"], scr["t4"]
    dump = scr["dump"]

    # ---- u = z - Y ----
    ue = v if U_ON == "dve" else gp
    ue.tensor_tensor(uA, zA, YA, op=OP.subtract)
    ue.tensor_tensor(us, zs, Ys, op=OP.subtract)
    ue.tensor_tensor(ub, zb, Yb, op=OP.subtract)

    # ---- mA = A^T uA ----
    a_v = cst["A"].rearrange("p (j i) -> p i j", j=m, i=n)
    t_v = tmp[:, 0:n * m].rearrange("p (i j) -> p i j", i=n, j=m)
    _emit_matvec_mult(nc, t_v, a_v, uA.unsqueeze(1).broadcast_to((P, n, m)),
                      DVI_AT, n)
    _emit_reduce(nc, mA, t_v, n, ATU_RED, dump)

    # ---- rhs (sigma terms dropped: sigma=1e-6 is below the noise floor) ----
    v.tensor_tensor(t80, mA, ub, op=OP.add)
    v.scalar_tensor_tensor(rhs[:, 0:n], t80, RHO, cst["x_raw"],
                           op0=OP.mult, op1=OP.add)
    v.tensor_tensor(t4, us, uA[:, m - ns:m], op=OP.subtract)
    v.tensor_scalar(rhs[:, n:N], t4, RHO, None, op0=OP.mult)

    # ---- xt = Kinv rhs ----
    kc = cst["Kinv"]  # column-major: [:, k*N:(k+1)*N] is column k
    if KINV_MODE == "stt":
        v.tensor_scalar(xt, kc[:, 0:N], rhs[:, 0:1], None, op0=OP.mult)
        for k in range(1, N):
            v.scalar_tensor_tensor(xt, kc[:, k * N:(k + 1) * N],
                                   rhs[:, k:k + 1], xt, op0=OP.mult, op1=OP.add)
    else:
        ki_v = kc.rearrange("p (k i) -> p i k", k=N, i=N)
        tk_v = tmp[:, 0:N * N].rearrange("p (i k) -> p i k", i=N, k=N)
        _emit_matvec_mult(nc, tk_v, ki_v,
                          rhs.unsqueeze(1).broadcast_to((P, N, N)),
                          N - KINV_GP_ROWS, N)
        v.tensor_reduce(xt, tk_v, axis=AX.X, op=OP.add)

    # ---- x' ----
    sc.mul(xts, xt, ALPHA)
    v.scalar_tensor_tensor(xn, x, 1.0 - ALPHA, xts, op0=OP.mult, op1=OP.add)

    # ---- ztA = A xt_x ----
    a_v2 = cst["A"].rearrange("p (j i) -> p j i", j=m, i=n)
    t_v2 = tmp[:, 0:m * n].rearrange("p (j i) -> p j i", j=m, i=n)
    _emit_matvec_mult(nc, t_v2, a_v2,
                      xt[:, 0:n].unsqueeze(1).broadcast_to((P, m, n)),
                      DVJ_AX, m)
    _emit_reduce(nc, ztA, t_v2, m, AXT_RED, dump)

    # ---- zr ----
    v.tensor_tensor(ztA[:, m - ns:m], ztA[:, m - ns:m], xt[:, n:N],
                    op=OP.subtract)
    sc.mul(ztAs, ztA, ALPHA)
    v.scalar_tensor_tensor(zrA, zA, 1.0 - ALPHA, ztAs, op0=OP.mult, op1=OP.add)
    v.scalar_tensor_tensor(zrs, zs, 1.0 - ALPHA, xts[:, n:N],
                           op0=OP.mult, op1=OP.add)
    v.scalar_tensor_tensor(zrb, zb, 1.0 - ALPHA, xts[:, 0:n],
                           op0=OP.mult, op1=OP.add)

    # ---- w = zr + Y (GPSIMD) ----
    gp.tensor_tensor(wA, zrA, YA, op=OP.add)
    gp.tensor_tensor(ws, zrs, Ys, op=OP.add)
    gp.tensor_tensor(wb, zrb, Yb, op=OP.add)

    # ---- zn = clip(w) ----
    v.tensor_tensor(zAn, wA, cst["b"], op=OP.min)
    sc.activation(zsn, ws, AF.Relu)
    v.tensor_tensor(t80, wb, cst["lo"], op=OP.max)
    v.tensor_tensor(zbn, t80, cst["up"], op=OP.min)

    # ---- Y' = w - zn (GPSIMD) ----
    gp.tensor_tensor(YAn, wA, zAn, op=OP.subtract)
    gp.tensor_tensor(Ysn, ws, zsn, op=OP.subtract)
    gp.tensor_tensor(Ybn, wb, zbn, op=OP.subtract)


def build_program(iters=ITERS):
    nc = bacc.Bacc("TRN2", target_bir_lowering=False, debug=False,
                   enable_asserts=False, num_devices=NCORES)

    A_d = nc.dram_tensor("a_bat", [T, P, m * n], f32, kind="ExternalInput").ap()
    Ki_d = nc.dram_tensor("kinv_bat", [T, P, N * N], f32,
                          kind="ExternalInput").ap()
    xr_d = nc.dram_tensor("xraw_bat", [T, P, n], f32, kind="ExternalInput").ap()
    b_d = nc.dram_tensor("b_bat", [T, P, m], f32, kind="ExternalInput").ap()
    lo_d = nc.dram_tensor("lo_bat", [T, P, n], f32, kind="ExternalInput").ap()
    up_d = nc.dram_tensor("up_bat", [T, P, n], f32, kind="ExternalInput").ap()
    xo_d = nc.dram_tensor("x_out", [T, P, n], f32, kind="ExternalOutput").ap()

    with TileContext(nc) as tc:
        with tc.tile_pool(name="main", bufs=1) as pool:
            tiles = []
            for t in range(T):
                cst = {}
                cst["A"] = pool.tile([P, m * n], f32, name=f"A_sb{t}")
                cst["Kinv"] = pool.tile([P, N * N], f32, name=f"Ki_sb{t}")
                cst["x_raw"] = pool.tile([P, n], f32, name=f"xr_sb{t}")
                cst["b"] = pool.tile([P, m], f32, name=f"b_sb{t}")
                cst["lo"] = pool.tile([P, n], f32, name=f"lo_sb{t}")
                cst["up"] = pool.tile([P, n], f32, name=f"up_sb{t}")
                nc.sync.dma_start(cst["A"], A_d[t])
                nc.sync.dma_start(cst["Kinv"], Ki_d[t])
                nc.sync.dma_start(cst["x_raw"], xr_d[t])
                nc.sync.dma_start(cst["b"], b_d[t])
                nc.sync.dma_start(cst["lo"], lo_d[t])
                nc.sync.dma_start(cst["up"], up_d[t])

                scr = {}
                scr["tmp"] = pool.tile([P, N * N], f32, name=f"tmp_{t}")
                for nm, w in (("uA", m), ("us", ns), ("ub", n), ("mA", n),
                              ("rhs", N), ("xt", N), ("xts", N), ("ztA", m),
                              ("ztAs", m), ("zrA", m), ("zrs", ns), ("zrb", n),
                              ("wA", m), ("ws", ns), ("wb", n), ("t80", n),
                              ("t4", ns), ("dump", m), ("touch", 8)):
                    scr[nm] = pool.tile([P, w], f32, name=f"{nm}_{t}")

                # absorb DMA queue-sem waits into engine clocks
                tch = scr["touch"]
                for i, key in enumerate(("A", "Kinv", "x_raw", "b", "lo",
                                         "up")):
                    nc.vector.tensor_scalar(tch[0:1, i:i + 1],
                                            cst[key][0:1, 0:1],
                                            1.0, None, op0=OP.mult)

                st = {}
                for nm, w in (("x", N), ("zA", m), ("zs", ns), ("zb", n),
                              ("YA", m), ("Ys", ns), ("Yb", n)):
                    st[nm] = pool.tile([P, w], f32, name=f"{nm}0_{t}")
                    st[nm + "_alt"] = pool.tile([P, w], f32, name=f"{nm}1_{t}")

                # ---- state init ----
                v = nc.vector
                v.tensor_scalar(st["x"][:, 0:n], cst["x_raw"], 1.0, None,
                                op0=OP.mult)
                v.memset(st["x"][:, n:N], 0.0)
                v.tensor_scalar(st["zb"], cst["x_raw"], 1.0, None, op0=OP.mult)
                v.memset(st["zs"], 0.0)
                v.memset(st["YA"], 0.0)
                v.memset(st["Ys"], 0.0)
                v.memset(st["Yb"], 0.0)
                a_v = cst["A"].rearrange("p (j i) -> p j i", j=m, i=n)
                t_v = scr["tmp"][:, 0:m * n].rearrange("p (j i) -> p j i",
                                                       j=m, i=n)
                v.tensor_tensor(t_v, a_v,
                                cst["x_raw"].unsqueeze(1).broadcast_to(
                                    (P, m, n)), op=OP.mult)
                v.tensor_reduce(st["zA"], t_v, axis=AX.X, op=OP.add)
                tiles.append((cst, st, scr))

            state_names = ("x", "zA", "zs", "zb", "YA", "Ys", "Yb")
            for it in range(iters):
                for t in range(T):
                    cst, st, scr = tiles[t]
                    a, bb = ("", "_alt") if it % 2 == 0 else ("_alt", "")
                    stmap = {nm: st[nm + a] for nm in state_names}
                    stmap.update({nm + "n": st[nm + bb] for nm in state_names})
                    _emit_iteration(nc, stmap, cst, scr)

            for t in range(T):
                cst, st, scr = tiles[t]
                final = st["x" + ("" if iters % 2 == 0 else "_alt")]
                nc.sync.dma_start(xo_d[t], final[:, 0:n])

    nc.finalize()
    return nc


_prog_cache = {}


def _get_prog(iters=ITERS):
    if iters not in _prog_cache:
        _prog_cache[iters] = build_program(iters)
    return _prog_cache[iters]


def _host_prep(inputs, core):
    sl = slice(core * PER_CORE, (core + 1) * PER_CORE)
    A = np.ascontiguousarray(inputs["A"][sl]).astype(np.float32)
    x_raw = np.ascontiguousarray(inputs["x_raw"][sl]).astype(np.float32)
    b = np.ascontiguousarray(inputs["b"][sl]).astype(np.float32)
    lo = np.ascontiguousarray(inputs["lower"][sl]).astype(np.float32)
    up = np.ascontiguousarray(inputs["upper"][sl]).astype(np.float32)

    B = PER_CORE
    As = A[:, -ns:]
    K = np.zeros((B, N, N), np.float32)
    K[:, :n, :n] = RHO * np.einsum("bmn,bmk->bnk", A, A) \
        + (RHO + 1 + SIGMA) * np.eye(n, dtype=np.float32)
    K[:, :n, n:] = -RHO * As.transpose(0, 2, 1)
    K[:, n:, :n] = -RHO * As
    K[:, n:, n:] = (2 * RHO + 2 * PEN + SIGMA) * np.eye(ns, dtype=np.float32)
    Kinv = np.linalg.inv(K.astype(np.float64)).astype(np.float32)
    # column-major per element (== row-major here: Kinv is symmetric, but
    # keep the transpose explicit for clarity)
    Kcm = np.ascontiguousarray(Kinv.transpose(0, 2, 1))

    return {
        "a_bat": A.reshape(T, P, m * n),
        "kinv_bat": Kcm.reshape(T, P, N * N),
        "xraw_bat": x_raw.reshape(T, P, n),
        "b_bat": b.reshape(T, P, m),
        "lo_bat": lo.reshape(T, P, n),
        "up_bat": up.reshape(T, P, n),
    }


def kernel(**inputs):
    nc = _get_prog()
    with ThreadPoolExecutor(NCORES) as ex:
        in_maps = list(ex.map(lambda c: _host_prep(inputs, c), range(NCORES)))
    res = run_bass_kernel_spmd(nc, in_maps, core_ids=list(range(NCORES)))
    outs = [r["x_out"].reshape(PER_CORE, n) for r in res.results]
    return np.concatenate(outs, axis=0).astype(inputs["x_raw"].dtype)


if __name__ == "__main__":
    rng = np.random.default_rng(0)
    fake = {
        "x_raw": rng.standard_normal((2048, n), dtype=np.float32),
        "A": rng.standard_normal((2048, m, n), dtype=np.float32) / np.sqrt(n),
        "b": rng.standard_normal((2048, m), dtype=np.float32),
        "lower": np.zeros((2048, n), np.float32),
        "upper": 100 * np.ones((2048, n), np.float32),
    }
    out = kernel(**fake)
    print(out.shape, out.dtype)


# revision 14
# speedup vs baseline: 1.0935x; 1.0935x over previous
"""Trainium2 Bass kernel for nn_CvxpyProjectionLayer.

Solves 2048 independent small QPs (projection with slack penalty) by an
OSQP-style ADMM, data-parallel over 8 NeuronCores (256 elements/core,
2 partition-tiles of 128 batch elements in partitions).

Algorithm notes:
 - The reference's 150 ADMM iterations (rho=1, alpha=1.6) are fully
   converged (ref@150 vs ref@600 < 1e-6), so we converge to the same fixed
   point faster: rho=2.0, alpha=1.7 hits the fp32 noise floor (~9e-5
   absmax vs the jax reference, full batch) in 42-46 iterations. The tiny
   sigma=1e-6 proximal terms are dropped (validated: no accuracy change).
 - KKT matrix K = rho*M^T M + diag(p)+sigma in closed form:
       K = [[rho*(A^T A) + (rho+1+sig)I , -rho*A_s^T],
            [-rho*A_s                   , (2rho+2+sig)I]]
 - Per-iteration batched matvecs (a different matrix per batch element)
   are spread over three engine rails:
     * A^T u, A xt: tensor_tensor multiply with broadcast APs, rows split
       between DVE and GPSIMD, + segmented tensor_reduce (DVE) or row-wise
       activation-accumulate reduce (ScalarE).
     * Kinv rhs: fused multiply-accumulate chain on DVE
       (scalar_tensor_tensor with per-partition scalars), one op per
       contraction index, reading contiguous columns of a column-major
       Kinv layout.
   Elementwise state updates go to GPSIMD/ScalarE/DVE.
"""

import sys
from concurrent.futures import ThreadPoolExecutor

import numpy as np

sys.path.insert(0, "/opt/trn_rl_repo")

import concourse.bacc as bacc  # noqa: E402
import concourse.mybir as mybir  # noqa: E402
from concourse.bass_utils import run_bass_kernel_spmd  # noqa: E402
from concourse.tile import TileContext  # noqa: E402

NCORES = 8
PER_CORE = 256
T = 2
P = 128
n = 80
m = 85
ns = 4
N = n + ns  # 84

RHO = 2.0
ALPHA = 1.7
SIGMA = 1e-6
PEN = 1.0
ITERS = 42

# --- engine assignment knobs (tuned against TimelineSim) ---
DVI_AT = 40        # A^T u mult: i-rows [0, DVI_AT) on DVE, rest GPSIMD
DVJ_AX = 42        # A xt mult: rows [0, DVJ_AX) on DVE, rest GPSIMD
ATU_RED = "dve"  # A^T u reduce: "dve" | "act"
AXT_RED = "dve"    # A xt reduce: "dve" | "act"
KINV_MODE = "mr"  # "stt": DVE MAC chain | "mr": mult(split)+reduce
KINV_GP_ROWS = 42   # when "mr": how many of the 84 i-rows multiply on GPSIMD

U_ON = "gp"

f32 = mybir.dt.float32
OP = mybir.AluOpType
AX = mybir.AxisListType
AF = mybir.ActivationFunctionType


def _emit_matvec_mult(nc, tmp_view, a_view, vec_bc, rows_dve, rows_total):
    """tmp[p, r, c] = a[p, r, c] * vec[p, c], rows split DVE/GPSIMD."""
    if rows_dve > 0:
        nc.vector.tensor_tensor(tmp_view[:, 0:rows_dve, :],
                                a_view[:, 0:rows_dve, :],
                                vec_bc[:, 0:rows_dve, :], op=OP.mult)
    if rows_dve < rows_total:
        nc.gpsimd.tensor_tensor(tmp_view[:, rows_dve:rows_total, :],
                                a_view[:, rows_dve:rows_total, :],
                                vec_bc[:, rows_dve:rows_total, :], op=OP.mult)


def _emit_reduce(nc, out, tmp_view, rows, where, dump):
    """out[p, r] = sum_c tmp[p, r, c], on DVE (one op) or ACT (row loop)."""
    if where == "dve":
        nc.vector.tensor_reduce(out, tmp_view, axis=AX.X, op=OP.add)
    else:
        for r in range(rows):
            nc.scalar.activation(dump[:, 0:tmp_view.shape[2]],
                                 tmp_view[:, r, :], AF.Copy,
                                 accum_out=out[:, r:r + 1])


def _emit_iteration(nc, st, cst, scr):
    v = nc.vector
    gp = nc.gpsimd
    sc = nc.scalar

    x, zA, zs, zb, YA, Ys, Yb = (
        st["x"], st["zA"], st["zs"], st["zb"], st["YA"], st["Ys"], st["Yb"])
    xn, zAn, zsn, zbn, YAn, Ysn, Ybn = (
        st["xn"], st["zAn"], st["zsn"], st["zbn"], st["YAn"], st["Ysn"], st["Ybn"])

    uA, us, ub = scr["uA"], scr["us"], scr["ub"]
    tmp = scr["tmp"]
    mA = scr["mA"]
    rhs = scr["rhs"]
    xt = scr["xt"]
    xts = scr["xts"]
    ztA = scr["ztA"]
    ztAs = scr["ztAs"]
    zrA, zrs, zrb = scr["zrA"], scr["zrs"], scr["zrb"]
    wA, ws, wb = scr["wA"], scr["ws"], scr["wb"]
    t80, t4 = scr["t80"], scr["t4"]
    dump = scr["dump"]

    # ---- u = z - Y ----
    ue = v if U_ON == "dve" else gp
    ue.tensor_tensor(uA, zA, YA, op=OP.subtract)
    ue.tensor_tensor(us, zs, Ys, op=OP.subtract)
    ue.tensor_tensor(ub, zb, Yb, op=OP.subtract)

    # ---- mA = A^T uA ----
    a_v = cst["A"].rearrange("p (j i) -> p i j", j=m, i=n)
    t_v = tmp[:, 0:n * m].rearrange("p (i j) -> p i j", i=n, j=m)
    _emit_matvec_mult(nc, t_v, a_v, uA.unsqueeze(1).broadcast_to((P, n, m)),
                      DVI_AT, n)
    _emit_reduce(nc, mA, t_v, n, ATU_RED, dump)

    # ---- rhs (sigma terms dropped: sigma=1e-6 is below the noise floor) ----
    v.tensor_tensor(t80, mA, ub, op=OP.add)
    v.scalar_tensor_tensor(rhs[:, 0:n], t80, RHO, cst["x_raw"],
                           op0=OP.mult, op1=OP.add)
    v.tensor_tensor(t4, us, uA[:, m - ns:m], op=OP.subtract)
    v.tensor_scalar(rhs[:, n:N], t4, RHO, None, op0=OP.mult)

    # ---- xt = Kinv rhs ----
    kc = cst["Kinv"]  # column-major: [:, k*N:(k+1)*N] is column k
    if KINV_MODE == "stt":
        v.tensor_scalar(xt, kc[:, 0:N], rhs[:, 0:1], None, op0=OP.mult)
        for k in range(1, N):
            v.scalar_tensor_tensor(xt, kc[:, k * N:(k + 1) * N],
                                   rhs[:, k:k + 1], xt, op0=OP.mult, op1=OP.add)
    else:
        ki_v = kc.rearrange("p (k i) -> p i k", k=N, i=N)
        tk_v = tmp[:, 0:N * N].rearrange("p (i k) -> p i k", i=N, k=N)
        _emit_matvec_mult(nc, tk_v, ki_v,
                          rhs.unsqueeze(1).broadcast_to((P, N, N)),
                          N - KINV_GP_ROWS, N)
        v.tensor_reduce(xt, tk_v, axis=AX.X, op=OP.add)

    # ---- x' ----
    sc.mul(xts, xt, ALPHA)
    v.scalar_tensor_tensor(xn, x, 1.0 - ALPHA, xts, op0=OP.mult, op1=OP.add)

    # ---- ztA = A xt_x ----
    a_v2 = cst["A"].rearrange("p (j i) -> p j i", j=m, i=n)
    t_v2 = tmp[:, 0:m * n].rearrange("p (j i) -> p j i", j=m, i=n)
    _emit_matvec_mult(nc, t_v2, a_v2,
                      xt[:, 0:n].unsqueeze(1).broadcast_to((P, m, n)),
                      DVJ_AX, m)
    _emit_reduce(nc, ztA, t_v2, m, AXT_RED, dump)

    # ---- zr ----
    v.tensor_tensor(ztA[:, m - ns:m], ztA[:, m - ns:m], xt[:, n:N],
                    op=OP.subtract)
    sc.mul(ztAs, ztA, ALPHA)
    v.scalar_tensor_tensor(zrA, zA, 1.0 - ALPHA, ztAs, op0=OP.mult, op1=OP.add)
    v.scalar_tensor_tensor(zrs, zs, 1.0 - ALPHA, xts[:, n:N],
                           op0=OP.mult, op1=OP.add)
    v.scalar_tensor_tensor(zrb, zb, 1.0 - ALPHA, xts[:, 0:n],
                           op0=OP.mult, op1=OP.add)

    # ---- w = zr + Y (GPSIMD) ----
    gp.tensor_tensor(wA, zrA, YA, op=OP.add)
    gp.tensor_tensor(ws, zrs, Ys, op=OP.add)
    gp.tensor_tensor(wb, zrb, Yb, op=OP.add)

    # ---- zn = clip(w) ----
    v.tensor_tensor(zAn, wA, cst["b"], op=OP.min)
    sc.activation(zsn, ws, AF.Relu)
    v.tensor_tensor(t80, wb, cst["lo"], op=OP.max)
    v.tensor_tensor(zbn, t80, cst["up"], op=OP.min)

    # ---- Y' = w - zn (GPSIMD) ----
    gp.tensor_tensor(YAn, wA, zAn, op=OP.subtract)
    gp.tensor_tensor(Ysn, ws, zsn, op=OP.subtract)
    gp.tensor_tensor(Ybn, wb, zbn, op=OP.subtract)


def build_program(iters=ITERS):
    nc = bacc.Bacc("TRN2", target_bir_lowering=False, debug=False,
                   enable_asserts=False, num_devices=NCORES)

    A_d = nc.dram_tensor("a_bat", [T, P, m * n], f32, kind="ExternalInput").ap()
    Ki_d = nc.dram_tensor("kinv_bat", [T, P, N * N], f32,
                          kind="ExternalInput").ap()
    xr_d = nc.dram_tensor("xraw_bat", [T, P, n], f32, kind="ExternalInput").ap()
    b_d = nc.dram_tensor("b_bat", [T, P, m], f32, kind="ExternalInput").ap()
    lo_d = nc.dram_tensor("lo_bat", [T, P, n], f32, kind="ExternalInput").ap()
    up_d = nc.dram_tensor("up_bat", [T, P, n], f32, kind="ExternalInput").ap()
    xo_d = nc.dram_tensor("x_out", [T, P, n], f32, kind="ExternalOutput").ap()

    with TileContext(nc) as tc:
        with tc.tile_pool(name="main", bufs=1) as pool:
            tiles = []
            for t in range(T):
                cst = {}
                cst["A"] = pool.tile([P, m * n], f32, name=f"A_sb{t}")
                cst["Kinv"] = pool.tile([P, N * N], f32, name=f"Ki_sb{t}")
                cst["x_raw"] = pool.tile([P, n], f32, name=f"xr_sb{t}")
                cst["b"] = pool.tile([P, m], f32, name=f"b_sb{t}")
                cst["lo"] = pool.tile([P, n], f32, name=f"lo_sb{t}")
                cst["up"] = pool.tile([P, n], f32, name=f"up_sb{t}")
                nc.sync.dma_start(cst["A"], A_d[t])
                nc.sync.dma_start(cst["Kinv"], Ki_d[t])
                nc.sync.dma_start(cst["x_raw"], xr_d[t])
                nc.sync.dma_start(cst["b"], b_d[t])
                nc.sync.dma_start(cst["lo"], lo_d[t])
                nc.sync.dma_start(cst["up"], up_d[t])

                scr = {}
                scr["tmp"] = pool.tile([P, N * N], f32, name=f"tmp_{t}")
                for nm, w in (("uA", m), ("us", ns), ("ub", n), ("mA", n),
                              ("rhs", N), ("xt", N), ("xts", N), ("ztA", m),
                              ("ztAs", m), ("zrA", m), ("zrs", ns), ("zrb", n),
                              ("wA", m), ("ws", ns), ("wb", n), ("t80", n),
                              ("t4", ns), ("dump", m), ("touch", 8)):
                    scr[nm] = pool.tile([P, w], f32, name=f"{nm}_{t}")

                # absorb DMA queue-sem waits into engine clocks
                tch = scr["touch"]
                for i, key in enumerate(("A", "Kinv", "x_raw", "b", "lo",
                                         "up")):
                    nc.vector.tensor_scalar(tch[0:1, i:i + 1],
                                            cst[key][0:1, 0:1],
                                            1.0, None, op0=OP.mult)

                st = {}
                for nm, w in (("x", N), ("zA", m), ("zs", ns), ("zb", n),
                              ("YA", m), ("Ys", ns), ("Yb", n)):
                    st[nm] = pool.tile([P, w], f32, name=f"{nm}0_{t}")
                    st[nm + "_alt"] = pool.tile([P, w], f32, name=f"{nm}1_{t}")

                # ---- state init ----
                v = nc.vector
                v.tensor_scalar(st["x"][:, 0:n], cst["x_raw"], 1.0, None,
                                op0=OP.mult)
                v.memset(st["x"][:, n:N], 0.0)
                v.tensor_scalar(st["zb"], cst["x_raw"], 1.0, None, op0=OP.mult)
                v.memset(st["zs"], 0.0)
                v.memset(st["YA"], 0.0)
                v.memset(st["Ys"], 0.0)
                v.memset(st["Yb"], 0.0)
                a_v = cst["A"].rearrange("p (j i) -> p j i", j=m, i=n)
                t_v = scr["tmp"][:, 0:m * n].rearrange("p (j i) -> p j i",
                                                       j=m, i=n)
                v.tensor_tensor(t_v, a_v,
                                cst["x_raw"].unsqueeze(1).broadcast_to(
                                    (P, m, n)), op=OP.mult)
                v.tensor_reduce(st["zA"], t_v, axis=AX.X, op=OP.add)
                tiles.append((cst, st, scr))

            state_names = ("x", "zA", "zs", "zb", "YA", "Ys", "Yb")
            for it in range(iters):
                for t in range(T):
                    cst, st, scr = tiles[t]
                    a, bb = ("", "_alt") if it % 2 == 0 else ("_alt", "")
                    stmap = {nm: st[nm + a] for nm in state_names}
                    stmap.update({nm + "n": st[nm + bb] for nm in state_names})
                    _emit_iteration(nc, stmap, cst, scr)

            for t in range(T):
                cst, st, scr = tiles[t]
                final = st["x" + ("" if iters % 2 == 0 else "_alt")]
                nc.sync.dma_start(xo_d[t], final[:, 0:n])

    nc.finalize()
    return nc


_prog_cache = {}


def _get_prog(iters=ITERS):
    if iters not in _prog_cache:
        _prog_cache[iters] = build_program(iters)
    return _prog_cache[iters]


def _host_prep(inputs, core):
    sl = slice(core * PER_CORE, (core + 1) * PER_CORE)
    A = np.ascontiguousarray(inputs["A"][sl]).astype(np.float32)
    x_raw = np.ascontiguousarray(inputs["x_raw"][sl]).astype(np.float32)
    b = np.ascontiguousarray(inputs["b"][sl]).astype(np.float32)
    lo = np.ascontiguousarray(inputs["lower"][sl]).astype(np.float32)
    up = np.ascontiguousarray(inputs["upper"][sl]).astype(np.float32)

    B = PER_CORE
    As = A[:, -ns:]
    K = np.zeros((B, N, N), np.float32)
    K[:, :n, :n] = RHO * np.einsum("bmn,bmk->bnk", A, A) \
        + (RHO + 1 + SIGMA) * np.eye(n, dtype=np.float32)
    K[:, :n, n:] = -RHO * As.transpose(0, 2, 1)
    K[:, n:, :n] = -RHO * As
    K[:, n:, n:] = (2 * RHO + 2 * PEN + SIGMA) * np.eye(ns, dtype=np.float32)
    Kinv = np.linalg.inv(K.astype(np.float64)).astype(np.float32)
    # column-major per element (== row-major here: Kinv is symmetric, but
    # keep the transpose explicit for clarity)
    Kcm = np.ascontiguousarray(Kinv.transpose(0, 2, 1))

    return {
        "a_bat": A.reshape(T, P, m * n),
        "kinv_bat": Kcm.reshape(T, P, N * N),
        "xraw_bat": x_raw.reshape(T, P, n),
        "b_bat": b.reshape(T, P, m),
        "lo_bat": lo.reshape(T, P, n),
        "up_bat": up.reshape(T, P, n),
    }


def kernel(**inputs):
    nc = _get_prog()
    with ThreadPoolExecutor(NCORES) as ex:
        in_maps = list(ex.map(lambda c: _host_prep(inputs, c), range(NCORES)))
    res = run_bass_kernel_spmd(nc, in_maps, core_ids=list(range(NCORES)))
    outs = [r["x_out"].reshape(PER_CORE, n) for r in res.results]
    return np.concatenate(outs, axis=0).astype(inputs["x_raw"].dtype)


if __name__ == "__main__":
    rng = np.random.default_rng(0)
    fake = {
        "x_raw": rng.standard_normal((2048, n), dtype=np.float32),
        "A": rng.standard_normal((2048, m, n), dtype=np.float32) / np.sqrt(n),
        "b": rng.standard_normal((2048, m), dtype=np.float32),
        "lower": np.zeros((2048, n), np.float32),
        "upper": 100 * np.ones((2048, n), np.float32),
    }
    out = kernel(**fake)
    print(out.shape, out.dtype)
